# revision 1
# baseline (speedup 1.0000x reference)
"""Trainium2 Bass kernel for nn_AttentionLayer (Luong attention, B=16, Te=Td=D=1024).

Full inputs in, full output out. Pure data-parallel over batch: 2 batches
per core on 8 NeuronCores.

Per batch (enc, dec are [1024, 1024] fp32):
  S[e, t]   = sum_d enc[e, d] * dec[t, d]          (fp16 matmul)
  E[e, t]   = exp(S - 160)                         (shift-invariant softmax:
                                                    global max ~215, smallest
                                                    col max ~87; exp(S-160)
                                                    spans [e^-87, e^55]: no
                                                    fp32 overflow, negligible
                                                    underflow)
  s[t]      = sum_e E[e, t]                        (ones-column matmul)
  V[t, d]   = (1/s[t]) * sum_e E[e, t] * enc[e, d]
  out       = [dec | V]

Design (v3; sim 136.5us vs the v1 baseline's 229.5us):
- The S matmul contracts over d, so enc/dec need d on partitions. Both
  are transposed ON-CHIP via PE transpose (identity matmul, fp16 1
  cycle/row, fp16 PSUM out) -> PSUM->SBUF copies (dh -> DVE, eh split
  ACT/DVE). No DRAM plane round trip: HBM traffic drops from v1's
  48MiB/core to 40MiB (24 in incl. the pass re-read + 16 out). PE is
  the bottleneck: S 54.6 + V 54.6 + transposes 13.6 + sums ~ 124us;
  model DMA ~82us.
- enc and dec are SWDGE cast-loaded fp32->fp16 straight into SBUF (no
  fp32 staging, no cast ops); the dec passthrough half of the output is
  a DRAM->DRAM copy that never touches SBUF.
- All loads sit dep-free on the single Pool/SWDGE ring in emission
  order: same-ring FIFO = back-to-back transfers with no sem gaps (the
  tile scheduler preserves dep-free same-ring order; explicit
  completion deps would add ~2.4us/link). Pass copies queue behind
  them, V stores go on the SP ring and interleave by arrival.
- The S phase is emitted in quarters: (i<4, j=0) needs only the g0
  (first-4-row-tiles) halves of ehT/dhT, so S compute starts ~12us in,
  right behind the first transposes, and per-matmul deps let each
  accumulation start before the last chunk's copy lands.
- Warm-up matmuls on a zero tile hold the PE p-state (0.65/1.2/2.4GHz,
  full speed only after 3us of continuous busy) through the brief
  load-bound head so real work never pays the ramp twice.
- Constants are tracked pool tiles built inside the TileContext (ident
  first on the Pool ring, memsets on DVE) instead of pre-context
  memsets + all-engine barrier, so the first load transfer starts
  ~1.6us in rather than ~3.9us.
"""
import sys

sys.path.insert(0, "/opt/trn_rl_repo")

import numpy as np

import concourse.bacc as bacc
import concourse.mybir as mybir
import concourse.tile as tile
from concourse.tile import add_dep_helper
from concourse.masks import make_identity
from concourse import bass_utils

F32 = mybir.dt.float32
F16 = mybir.dt.float16
BF16 = mybir.dt.bfloat16
AF = mybir.ActivationFunctionType

P = 128          # partitions
NB = 2           # batches per core
T = 1024         # Te = Td
D = 1024
KT = T // P      # 8 row-tiles per matrix
HG = 4           # row-tiles per load group (2 groups per matrix)
NC = 8           # cores
SHIFT = -160.0
# warm-up matmul counts ([P,512] fp16, ~213ns each) filling PE head gaps:
# before T_dh(g0) / between T_dh g0-g1 / T_dh g1-T_eh g0 / T_eh g0-g1
WARMS = (8, 4, 0, 0)

_CACHED = {}


def build_kernel(warms=WARMS):
    nc = bacc.Bacc("TRN2", target_bir_lowering=False, debug=False, num_devices=NC)

    enc_d = nc.dram_tensor("encoder_outputs", [NB * T, D], F32, kind="ExternalInput")
    dec_d = nc.dram_tensor("decoder_outputs", [NB * T, D], F32, kind="ExternalInput")
    out_d = nc.dram_tensor("out", [NB * T, 2 * D], F32, kind="ExternalOutput")

    with tile.TileContext(nc) as tc:
        with (
            tc.tile_pool(name="const", bufs=1) as p_const,
            tc.tile_pool(name="eh", bufs=1) as p_eh,
            tc.tile_pool(name="dh", bufs=1) as p_dh,
            tc.tile_pool(name="ehT", bufs=1) as p_ehT,
            tc.tile_pool(name="dhT", bufs=1) as p_dhT,
            tc.tile_pool(name="E", bufs=1) as p_E,
            tc.tile_pool(name="vout", bufs=4) as p_vout,
            tc.tile_pool(name="small", bufs=16) as p_small,
            tc.tile_pool(name="ps_tr", bufs=3, space="PSUM") as ps_tr,
            tc.tile_pool(name="ps_s", bufs=2, space="PSUM") as ps_s,
            tc.tile_pool(name="ps_v", bufs=2, space="PSUM") as ps_v,
            tc.tile_pool(name="ps_sum", bufs=1, space="PSUM") as ps_sum,
        ):
            warm_src = p_const.tile([P, 512], F16, tag="wsrc", name="wsrc")
            nc.vector.memset(warm_src[:], 0.0)
            ident_t = p_const.tile([P, P], F16, tag="ident", name="ident")
            make_identity(nc, ident_t[:])
            ones16 = p_const.tile([P, 1], F16, tag="ones", name="ones")
            nc.vector.memset(ones16[:], 1.0)
            bias_sh = p_const.tile([P, 1], F32, tag="bias", name="bias")
            nc.vector.memset(bias_sh[:], SHIFT)

            st = {"eh": {}, "dh": {}, "ehT": {}, "dhT": {}, "E": {}}

            def dram_rows(dram, b, g, cols):
                rows = dram.ap()[b * T + g * HG * P: b * T + (g + 1) * HG * P, cols]
                return rows.rearrange("(i p) d -> p i d", p=P)

            def load_dh(b, g):
                # SWDGE cast-load: dec fp32 HBM -> dh fp16 SBUF (no fp32
                # staging; the passthrough goes DRAM->DRAM separately)
                t = p_dh.tile([P, HG, D], F16, tag=f"dh{b}{g}", name=f"dh{b}{g}")
                nc.gpsimd.dma_start(t[:], dram_rows(dec_d, b, g, slice(None)))
                st["dh"][b, g] = t

            def load_enc(b, g, after=None):
                t = p_eh.tile([P, HG, D], F16, tag=f"eh{b}{g}", name=f"eh{b}{g}")
                # SWDGE cast-load: fp32 HBM -> fp16 SBUF
                nc.gpsimd.dma_start(t[:], dram_rows(enc_d, b, g, slice(None)))
                inst = nc.cur_bb.bb.instructions[-1]
                if after is not None:
                    add_dep_helper(inst, after, reason="serialize load chain")
                st["eh"][b, g] = t
                return inst

            def store_pass(b, g):
                # dec passthrough: DRAM -> DRAM, no SBUF staging; emitted
                # after the loads on the same SWDGE ring so FIFO keeps it
                # out of the pipeline-critical load window
                nc.gpsimd.dma_start(dram_rows(out_d, b, g, slice(0, D)),
                                    dram_rows(dec_d, b, g, slice(None)))

            def warm(n):
                if n <= 0:
                    return
                wps = ps_v.tile([P, 512], F32, tag="vps", name="warm")
                for _ in range(n):
                    nc.tensor.matmul(wps[:], warm_src[:, 0:P],
                                     warm_src[:], start=True, stop=True)

            def t_group(b, mat, g):
                """PE-transpose tiles g*HG..g*HG+3 of eh/dh into [mat]T chunk
                halves; copies split Pool (dh) / ACT (eh)."""
                src = st[mat][b, g]
                dstmap, pool = (st["dhT"], p_dhT) if mat == "dh" else (st["ehT"], p_ehT)
                for k in range(KT):
                    trp = ps_tr.tile([P, 512], F16, tag="tr", name="tr")
                    for q in range(HG):
                        nc.tensor.matmul(
                            trp[:, q * P:(q + 1) * P],
                            src[:, q, k * P:(k + 1) * P],
                            ident_t[:],
                            is_transpose=True, start=True, stop=True,
                        )
                    if g == 0:
                        dstmap[b, k] = pool.tile([P, T], F16, tag=f"{mat}T{k}",
                                                 name=f"{mat}T{b}{k}")
                    dst = dstmap[b, k][:, g * 512:(g + 1) * 512]
                    # dh copies -> DVE; eh copies split ACT/DVE so neither
                    # engine's serial stream gates the S start
                    if mat == "dh":
                        nc.vector.tensor_copy(dst, trp[:])
                    elif k < 3:
                        nc.scalar.activation(dst, trp[:], AF.Copy)
                    else:
                        nc.vector.tensor_copy(dst, trp[:])

            def s_quarter(b, i_range, j):
                """One quarter of S: e-tiles i_range x t-chunk j. Quarter
                (i<4, j=0) only needs the g0 halves of ehT/dhT, so it can
                start before the g1 loads/transposes land."""
                ehT, dhT = st["ehT"], st["dhT"]
                for i in i_range:
                    sps = ps_s.tile([P, 512], F32, tag="sps", name="sps")
                    for k in range(KT):
                        nc.tensor.matmul(
                            sps[:],
                            ehT[b, k][:, i * P:(i + 1) * P],
                            dhT[b, k][:, j * 512:(j + 1) * 512],
                            start=(k == 0), stop=(k == KT - 1),
                        )
                    if (b, i) not in st["E"]:
                        st["E"][b, i] = p_E.tile([P, T], BF16, tag=f"E{i}",
                                                 name=f"E{b}{i}")
                    nc.scalar.activation(st["E"][b, i][:, j * 512:(j + 1) * 512],
                                         sps[:], AF.Exp, bias=bias_sh[:],
                                         scale=1.0)

            def s_phase(b):
                s_quarter(b, range(4), 0)
                s_quarter(b, range(4), 1)
                s_quarter(b, range(4, 8), 0)
                s_quarter(b, range(4, 8), 1)

            def v_phase(b):
                E, eh = st["E"], st["eh"]
                for m in range(KT):
                    msl = slice(m * P, (m + 1) * P)
                    ssp = ps_sum.tile([P, 1], F32, tag="ssp", name="ssp")
                    for k in range(KT):
                        nc.tensor.matmul(ssp[:], E[b, k][:, msl], ones16[:],
                                         start=(k == 0), stop=(k == KT - 1))
                    r = p_small.tile([P, 1], F32, tag="r", name="r")
                    nc.vector.reciprocal(r[:], ssp[:])
                    last = (b == NB - 1 and m == KT - 1)
                    nh = 4 if last else 2
                    for h in range(nh):
                        w = 1024 // nh
                        hsl = slice(h * w, (h + 1) * w)
                        vps = ps_v.tile([P, w], F32, tag="vps", name="vps")
                        for k in range(KT):
                            nc.tensor.matmul(vps[:], E[b, k][:, msl],
                                             eh[b, k // HG][:, k % HG, hsl],
                                             start=(k == 0), stop=(k == KT - 1))
                        vsb = p_vout.tile([P, w], F32, tag="vsb", name="vsb")
                        nc.vector.tensor_scalar_mul(vsb[:], vps[:], r[:])
                        nc.sync.dma_start(
                            out_d.ap()[b * T + m * P: b * T + (m + 1) * P,
                                       D + h * w: D + (h + 1) * w],
                            vsb[:],
                        )

            # --- loads: all on the Pool/SWDGE ring, dep-free, in emission
            # order (same-ring FIFO = back-to-back transfers, no sem gaps) ---
            load_dh(0, 0)
            load_enc(0, 0)
            load_dh(0, 1)
            load_enc(0, 1)
            load_dh(1, 0)
            load_enc(1, 0)
            load_dh(1, 1)
            load_enc(1, 1)
            # dec passthrough behind the loads on the same FIFO ring
            store_pass(0, 0)
            store_pass(0, 1)
            store_pass(1, 0)
            store_pass(1, 1)

            # --- PE program (emission order = PE order); casts emitted
            # where the DVE ring order needs them ---
            warm(warms[0])
            t_group(0, "dh", 0)
            warm(warms[1])
            t_group(0, "eh", 0)
            warm(warms[2])
            s_quarter(0, range(4), 0)       # needs only g0 halves
            t_group(0, "dh", 1)
            t_group(0, "eh", 1)
            s_quarter(0, range(4), 1)
            s_quarter(0, range(4, 8), 0)
            s_quarter(0, range(4, 8), 1)
            t_group(1, "dh", 0)
            t_group(1, "eh", 0)
            t_group(1, "dh", 1)
            t_group(1, "eh", 1)
            v_phase(0)
            s_phase(1)
            v_phase(1)



    nc.compile()
    return nc


def kernel(encoder_outputs: np.ndarray, decoder_outputs: np.ndarray) -> np.ndarray:
    enc = np.ascontiguousarray(encoder_outputs, dtype=np.float32)
    dec = np.ascontiguousarray(decoder_outputs, dtype=np.float32)
    B = enc.shape[0]
    bpc = B // NC  # batches per core

    if "nc" not in _CACHED:
        _CACHED["nc"] = build_kernel()
    nc = _CACHED["nc"]

    in_maps = [
        {
            "encoder_outputs": enc[c * bpc:(c + 1) * bpc].reshape(NB * T, D),
            "decoder_outputs": dec[c * bpc:(c + 1) * bpc].reshape(NB * T, D),
        }
        for c in range(NC)
    ]
    res = bass_utils.run_bass_kernel_spmd(nc, in_maps, core_ids=list(range(NC)))
    out = np.concatenate(
        [res.results[c]["out"].reshape(bpc, T, 2 * D) for c in range(NC)], axis=0
    )
    return out



# revision 17
# speedup vs baseline: 65.8015x; 65.8015x over previous
"""Trainium2 Bass kernel for nn_AttentionLayer (Luong attention, B=16, Te=Td=D=1024).

Full inputs in, full output out. Pure data-parallel over batch: 2 batches
per core on 8 NeuronCores.

Per batch (enc, dec are [1024, 1024] fp32):
  S[e, t]   = sum_d enc[e, d] * dec[t, d]          (fp16 matmul)
  E[e, t]   = exp(S - 160)                         (shift-invariant softmax:
                                                    global max ~215, smallest
                                                    col max ~87; exp(S-160)
                                                    spans [e^-87, e^55]: no
                                                    fp32 overflow, negligible
                                                    underflow)
  s[t]      = sum_e E[e, t]                        (ones-column matmul)
  V[t, d]   = (1/s[t]) * sum_e E[e, t] * enc[e, d]
  out       = [dec | V]

Design (v4; NTFF-measured 157-162us vs the v1 baseline's 184-193us; the
device clock floats run-to-run between ~2.0 and ~2.4 GHz, so absolute
times swing ~20% with it):
- The S matmul contracts over d, so enc/dec need d on partitions. Both
  are transposed ON-CHIP via PE transpose (identity matmul, fp16 1
  cycle/row, fp16 PSUM out) -> PSUM->SBUF copies alternating DVE/ACT.
  (DMA-XBAR transpose was tried and is bit-exact, but the framework
  serializes every DMA-transpose against ALL in-flight DMA - deadlock
  guard - which made the load stream 2.4x slower; PE transposes win.)
- enc and dec are SWDGE cast-loaded fp32->fp16 straight into SBUF; the
  dec passthrough half of the output is a SWDGE cast-store fp16->f32
  from the dh tiles, queued on the load ring after all loads. That
  saves the 8MiB/core DRAM re-read a DRAM->DRAM pass copy would do
  (~27us of measured ring time) and costs only fp16 rounding of the
  dec half (~2.4e-4 rel err against the 2e-2 gate). HBM traffic:
  32MiB in + 16MiB out per core, streaming at ~420-500GB/s measured.
- All loads sit dep-free on the single Pool/SWDGE ring in emission
  order (same-ring FIFO = back-to-back transfers, no sem gaps); V
  stores go on the SP ring and interleave by arrival.
- The V phase is k-outer with ONE explicit LDWEIGHTS of each E chunk
  feeding three non-self-loading matmuls (ldweights=False): the
  colsum (ones moving) and both 512-wide V chunks. The colsum matmuls
  ride along nearly free (~40ns each).
- The S phase is emitted in quarters: (i<4, j=0) needs only the g0
  halves of ehT/dhT, so S compute starts ~25us in, right behind the
  first transposes; b1's transposes fill the PE between S0 and V0.
- Warm-up matmuls hold the PE p-state (full speed needs ~3us of
  continuous busy; a >5us idle drops it to half speed k=4/8) through
  the load-bound head: 8 before the first transposes, 16 across the
  eh00 load wait.
- 512-col fp16 matmuls retire at N/clk + ~3ns (216ns @2.4GHz) with
  LDWEIGHTS fully hidden by the PE's reorder window; the kernel is
  PE-bound from ~25us on, so the remaining span is S (2x27.6) +
  V (2x30) + transposes/copies (~28) + head/teardown (~18).
"""
import sys

sys.path.insert(0, "/opt/trn_rl_repo")

import numpy as np

import concourse.bacc as bacc
import concourse.mybir as mybir
import concourse.tile as tile
from concourse.tile import add_dep_helper
from concourse.masks import make_identity
from concourse import bass_utils

F32 = mybir.dt.float32
F16 = mybir.dt.float16
BF16 = mybir.dt.bfloat16
AF = mybir.ActivationFunctionType

P = 128          # partitions
NB = 2           # batches per core
T = 1024         # Te = Td
D = 1024
KT = T // P      # 8 row-tiles per matrix
HG = 4           # row-tiles per load group (2 groups per matrix)
NC = 8           # cores
SHIFT = -160.0
# warm-up matmul counts ([P,512] fp16, ~216-259ns each) filling PE head
# gaps: before T_dh(g0) / the eh00 load wait after T_dh g0 / unused x2
WARMS = (8, 16, 0, 0)

_CACHED = {}


def build_kernel(warms=WARMS):
    nc = bacc.Bacc("TRN2", target_bir_lowering=False, debug=False, num_devices=NC)

    enc_d = nc.dram_tensor("encoder_outputs", [NB * T, D], F32, kind="ExternalInput")
    dec_d = nc.dram_tensor("decoder_outputs", [NB * T, D], F32, kind="ExternalInput")
    out_d = nc.dram_tensor("out", [NB * T, 2 * D], F32, kind="ExternalOutput")

    with tile.TileContext(nc) as tc:
        with (
            tc.tile_pool(name="const", bufs=1) as p_const,
            tc.tile_pool(name="eh", bufs=1) as p_eh,
            tc.tile_pool(name="dh", bufs=1) as p_dh,
            tc.tile_pool(name="ehT", bufs=1) as p_ehT,
            tc.tile_pool(name="dhT", bufs=1) as p_dhT,
            tc.tile_pool(name="E", bufs=1) as p_E,
            tc.tile_pool(name="vout", bufs=4) as p_vout,
            tc.tile_pool(name="small", bufs=16) as p_small,
            tc.tile_pool(name="ps_tr", bufs=2, space="PSUM") as ps_tr,
            tc.tile_pool(name="ps_s", bufs=2, space="PSUM") as ps_s,
            tc.tile_pool(name="ps_v", bufs=3, space="PSUM") as ps_v,
            tc.tile_pool(name="ps_sum", bufs=1, space="PSUM") as ps_sum,
        ):
            warm_src = p_const.tile([P, 512], F16, tag="wsrc", name="wsrc")
            nc.vector.memset(warm_src[:], 0.0)
            ident_t = p_const.tile([P, P], F16, tag="ident", name="ident")
            make_identity(nc, ident_t[:])
            ones16 = p_const.tile([P, 1], F16, tag="ones", name="ones")
            nc.vector.memset(ones16[:], 1.0)
            bias_sh = p_const.tile([P, 1], F32, tag="bias", name="bias")
            nc.vector.memset(bias_sh[:], SHIFT)

            st = {"eh": {}, "dh": {}, "ehT": {}, "dhT": {}, "E": {}}

            def dram_rows(dram, b, g, cols):
                rows = dram.ap()[b * T + g * HG * P: b * T + (g + 1) * HG * P, cols]
                return rows.rearrange("(i p) d -> p i d", p=P)

            def load_dh(b, g):
                # SWDGE cast-load: dec fp32 HBM -> dh fp16 SBUF (no fp32
                # staging; the passthrough goes DRAM->DRAM separately)
                t = p_dh.tile([P, HG, D], F16, tag=f"dh{b}{g}", name=f"dh{b}{g}")
                nc.gpsimd.dma_start(t[:], dram_rows(dec_d, b, g, slice(None)))
                st["dh"][b, g] = t

            def load_enc(b, g, after=None):
                t = p_eh.tile([P, HG, D], F16, tag=f"eh{b}{g}", name=f"eh{b}{g}")
                # SWDGE cast-load: fp32 HBM -> fp16 SBUF
                nc.gpsimd.dma_start(t[:], dram_rows(enc_d, b, g, slice(None)))
                inst = nc.cur_bb.bb.instructions[-1]
                if after is not None:
                    add_dep_helper(inst, after, reason="serialize load chain")
                st["eh"][b, g] = t
                return inst

            def store_pass(b, g):
                # dec passthrough: SWDGE cast-store fp16 SBUF -> f32 DRAM
                # from the already-loaded dh tile. Saves the 8MiB/core DRAM
                # re-read of a DRAM->DRAM copy (~27us of measured ring
                # time); fp16-rounding the dec half adds ~2.4e-4 rel err
                # against the 2e-2 gate.
                nc.gpsimd.dma_start(dram_rows(out_d, b, g, slice(0, D)),
                                    st["dh"][b, g][:])

            def warm(n):
                if n <= 0:
                    return
                wps = ps_v.tile([P, 512], F32, tag="vps", name="warm")
                for _ in range(n):
                    nc.tensor.matmul(wps[:], warm_src[:, 0:P],
                                     warm_src[:], start=True, stop=True)

            def t_group(b, mat, g):
                """PE-transpose tiles g*HG..g*HG+3 of eh/dh into [mat]T chunk
                halves; copies alternate DVE/ACT."""
                src = st[mat][b, g]
                dstmap, pool = (st["dhT"], p_dhT) if mat == "dh" else (st["ehT"], p_ehT)
                for k in range(KT):
                    trp = ps_tr.tile([P, 512], F16, tag="tr", name="tr")
                    for q in range(HG):
                        nc.tensor.matmul(
                            trp[:, q * P:(q + 1) * P],
                            src[:, q, k * P:(k + 1) * P],
                            ident_t[:],
                            is_transpose=True, start=True, stop=True,
                        )
                    if g == 0:
                        dstmap[b, k] = pool.tile([P, T], F16, tag=f"{mat}T{k}",
                                                 name=f"{mat}T{b}{k}")
                    dst = dstmap[b, k][:, g * 512:(g + 1) * 512]
                    # PSUM->SBUF copies alternate DVE/ACT (GPSIMD cannot
                    # read PSUM) so neither serial stream gates transposes
                    if k % 2 == 0:
                        nc.vector.tensor_copy(dst, trp[:])
                    else:
                        nc.scalar.activation(dst, trp[:], AF.Copy)

            def s_quarter(b, i_range, j):
                """One quarter of S: e-tiles i_range x t-chunk j. Quarter
                (i<4, j=0) only needs the g0 halves of ehT/dhT, so it can
                start before the g1 loads/transposes land."""
                ehT, dhT = st["ehT"], st["dhT"]
                for i in i_range:
                    sps = ps_s.tile([P, 512], F32, tag="sps", name="sps")
                    for k in range(KT):
                        nc.tensor.matmul(
                            sps[:],
                            ehT[b, k][:, i * P:(i + 1) * P],
                            dhT[b, k][:, j * 512:(j + 1) * 512],
                            start=(k == 0), stop=(k == KT - 1),
                        )
                    if (b, i) not in st["E"]:
                        st["E"][b, i] = p_E.tile([P, T], BF16, tag=f"E{i}",
                                                 name=f"E{b}{i}")
                    nc.scalar.activation(st["E"][b, i][:, j * 512:(j + 1) * 512],
                                         sps[:], AF.Exp, bias=bias_sh[:],
                                         scale=1.0)

            def s_phase(b):
                s_quarter(b, range(4), 0)
                s_quarter(b, range(4), 1)
                s_quarter(b, range(4, 8), 0)
                s_quarter(b, range(4, 8), 1)

            def v_phase(b):
                E, eh = st["E"], st["eh"]
                for m in range(KT):
                    msl = slice(m * P, (m + 1) * P)
                    ssp = ps_sum.tile([P, 1], F32, tag="ssp", name="ssp")
                    vps0 = ps_v.tile([P, 512], F32, tag="vps", name="vps")
                    vps1 = ps_v.tile([P, 512], F32, tag="vps", name="vps")
                    # k-outer: one explicit LDWEIGHTS of the E chunk feeds
                    # the colsum and both 512-wide V chunks (the two extra
                    # weight loads per (m, k) are elided)
                    for k in range(KT):
                        nc.tensor.ldweights(E[b, k][:, msl])
                        nc.tensor.matmul(ssp[:], E[b, k][:, msl], ones16[:],
                                         start=(k == 0), stop=(k == KT - 1))
                        nc.cur_bb.bb.instructions[-1].ldweights = False
                        nc.tensor.matmul(vps0[:], E[b, k][:, msl],
                                         eh[b, k // HG][:, k % HG, 0:512],
                                         start=(k == 0), stop=(k == KT - 1))
                        nc.cur_bb.bb.instructions[-1].ldweights = False
                        nc.tensor.matmul(vps1[:], E[b, k][:, msl],
                                         eh[b, k // HG][:, k % HG, 512:1024],
                                         start=(k == 0), stop=(k == KT - 1))
                        nc.cur_bb.bb.instructions[-1].ldweights = False
                    r = p_small.tile([P, 1], F32, tag="r", name="r")
                    nc.vector.reciprocal(r[:], ssp[:])
                    for h, vps in ((0, vps0), (1, vps1)):
                        vsb = p_vout.tile([P, 512], F32, tag="vsb", name="vsb")
                        nc.vector.tensor_scalar_mul(vsb[:], vps[:], r[:])
                        nc.sync.dma_start(
                            out_d.ap()[b * T + m * P: b * T + (m + 1) * P,
                                       D + h * 512: D + (h + 1) * 512],
                            vsb[:],
                        )

            # --- loads: all on the Pool/SWDGE ring, dep-free, in emission
            # order (same-ring FIFO = back-to-back transfers, no sem gaps) ---
            load_dh(0, 0)
            load_enc(0, 0)
            load_dh(0, 1)
            load_enc(0, 1)
            load_dh(1, 0)
            load_enc(1, 0)
            load_dh(1, 1)
            load_enc(1, 1)
            # dec passthrough behind the loads on the same FIFO ring
            store_pass(0, 0)
            store_pass(0, 1)
            store_pass(1, 0)
            store_pass(1, 1)

            # --- PE program (emission order = PE order) ---
            warm(warms[0])
            t_group(0, "dh", 0)
            warm(warms[1])
            t_group(0, "eh", 0)
            warm(warms[2])
            s_quarter(0, range(4), 0)       # needs only g0 halves
            t_group(0, "dh", 1)
            t_group(0, "eh", 1)
            s_quarter(0, range(4), 1)
            s_quarter(0, range(4, 8), 0)
            s_quarter(0, range(4, 8), 1)
            t_group(1, "dh", 0)
            t_group(1, "eh", 0)
            t_group(1, "dh", 1)
            t_group(1, "eh", 1)
            v_phase(0)
            s_phase(1)
            v_phase(1)



    nc.compile()
    return nc


def kernel(encoder_outputs: np.ndarray, decoder_outputs: np.ndarray) -> np.ndarray:
    enc = np.ascontiguousarray(encoder_outputs, dtype=np.float32)
    dec = np.ascontiguousarray(decoder_outputs, dtype=np.float32)
    B = enc.shape[0]
    bpc = B // NC  # batches per core

    if "nc" not in _CACHED:
        _CACHED["nc"] = build_kernel()
    nc = _CACHED["nc"]

    in_maps = [
        {
            "encoder_outputs": enc[c * bpc:(c + 1) * bpc].reshape(NB * T, D),
            "decoder_outputs": dec[c * bpc:(c + 1) * bpc].reshape(NB * T, D),
        }
        for c in range(NC)
    ]
    res = bass_utils.run_bass_kernel_spmd(nc, in_maps, core_ids=list(range(NC)))
    out = np.concatenate(
        [res.results[c]["out"].reshape(bpc, T, 2 * D) for c in range(NC)], axis=0
    )
    return out

